# revision 18
# baseline (speedup 1.0000x reference)
"""Multi-head attention (B=2, S=2048, H=1024, 16 heads) on 8 TRN2 NeuronCores.

Sharding: data parallel on batch (2) x tensor parallel on heads (4 heads/core,
Megatron column-split qkv, row-split wo). Host pre-transposes x/y, pre-scales
wq by dh^-0.5, and sum-reduces the 4 partial outputs per batch element.

Per-core kernel (v2):
  The attention loop is ACT(exp)-paced at ~1.1us/step (128 steps). All
  projection work beyond a minimal prefix (K/Q j-block 0 + first two V
  subtiles) is woven into the loop via a deadline-sorted schedule, so the
  prefix shrinks from ~59us to ~12us. Input DMA is chunked [128,512] and
  ordered so the first projection group can start as chunks land; dummy
  warmup matmuls ramp the PE p-state during the initial DMA wait. Output
  stays f32 (bf16 DRAM outputs do not land on HW).

  Attention per 512-wide q-block and head pair: row-tiled (2-head packed)
  QK^T -> logitsT psum [128,1024] -> one ACT exp per pair (psum->sbuf bf16)
  -> PV matmul with fused denominator row (fp32 accumulate) -> fast
  reciprocal + gpsimd partition_broadcast normalize -> pair-stacked bf16
  output projection, deferred one block for overlap.
"""
import sys
sys.path.insert(0, '/opt/trn_rl_repo')
from contextlib import ExitStack

import numpy as np
import ml_dtypes

import concourse.bacc as bacc
import concourse.tile as tile
from concourse import mybir
from concourse import bass_utils

B, S, H, NH = 2, 2048, 1024, 16
DH = H // NH            # 64
NCORES = 8
HPC = NH // (NCORES // B)   # 4 heads per core
C = HPC * DH            # 256 projected cols per core
KT_H = H // 128         # 8 contraction tiles over H
SK = S // 128           # 16 s-subtiles
JBLK = 512
NJ = S // JBLK          # 4 q-blocks
F32 = mybir.dt.float32
BF16 = mybir.dt.bfloat16

_CACHE = {}
_DEBUG = False


def _build():
    nc = bacc.Bacc('TRN2', debug=False, num_devices=NCORES)
    xT = nc.dram_tensor('xT', [H, S], BF16, kind='ExternalInput')
    yT = nc.dram_tensor('yT', [H, S], BF16, kind='ExternalInput')
    wq = nc.dram_tensor('wq', [H, C], BF16, kind='ExternalInput')
    wk = nc.dram_tensor('wk', [H, C], BF16, kind='ExternalInput')
    wv = nc.dram_tensor('wv', [H, C], BF16, kind='ExternalInput')
    wo = nc.dram_tensor('wo', [C, H], BF16, kind='ExternalInput')
    ebias = nc.dram_tensor('ebias', [128, SK], F32, kind='ExternalInput')
    out = nc.dram_tensor('out', [S, H], F32, kind='ExternalOutput')
    dbg = {}
    if _DEBUG:
        for nm, w in [('d_kt', S), ('d_qt', S), ('d_ex', 2 * JBLK),
                      ('d_v', HPC * (DH + 1)), ('d_wo', 2 * H),
                      ('d_ctx', JBLK), ('d_raw', JBLK)]:
            dbg[nm] = nc.dram_tensor(nm, [128, w], F32, kind='ExternalOutput')

    with tile.TileContext(nc) as tc, ExitStack() as ctx:
        res = ctx.enter_context(tc.tile_pool(name='res', bufs=1))
        expool = ctx.enter_context(tc.tile_pool(name='expool', bufs=4))
        ctxpool = ctx.enter_context(tc.tile_pool(name='ctxpool', bufs=2))
        small = ctx.enter_context(tc.tile_pool(name='small', bufs=3))
        outpool = ctx.enter_context(tc.tile_pool(name='outpool', bufs=3))
        ps_qk = ctx.enter_context(tc.tile_pool(name='ps_qk', bufs=2, space='PSUM'))
        ps_acc = ctx.enter_context(tc.tile_pool(name='ps_acc', bufs=2, space='PSUM'))
        ps_g = ctx.enter_context(tc.tile_pool(name='ps_g', bufs=2, space='PSUM'))

        # ---- input DMAs, ordered for earliest compute start ----
        eb = res.tile([128, SK], F32, tag='eb')
        wk_r = res.tile([128, KT_H, C], BF16, tag='wk')
        wv_r = res.tile([128, KT_H, C], BF16, tag='wv')
        wq_r = res.tile([128, KT_H, C], BF16, tag='wq')
        wo_r = res.tile([128, 2, H], BF16, tag='wo')
        nc.sync.dma_start(out=wk_r, in_=wk.ap().rearrange('(t p) c -> p t c', p=128))
        nc.sync.dma_start(out=eb, in_=ebias.ap())
        WVQ_DMA = [lambda: (
            nc.sync.dma_start(out=wv_r,
                              in_=wv.ap().rearrange('(t p) c -> p t c', p=128)),
            nc.sync.dma_start(out=wq_r,
                              in_=wq.ap().rearrange('(t p) c -> p t c', p=128)))]

        xrows = [res.tile([128, S], BF16, tag=f'xr{k}', name=f'xr{k}')
                 for k in range(KT_H)]
        yrows = [res.tile([128, S], BF16, tag=f'yr{k}', name=f'yr{k}')
                 for k in range(KT_H)]
        xts = [[xrows[k][:, j * JBLK:(j + 1) * JBLK] for j in range(NJ)]
               for k in range(KT_H)]
        yts = [[yrows[k][:, j * JBLK:(j + 1) * JBLK] for j in range(NJ)]
               for k in range(KT_H)]
        xT_ap, yT_ap = xT.ap(), yT.ap()

        # each dma_start costs ~600ns of ISSUING-sequencer time: keep the
        # count small (full rows) and keep ALL of them off the ACT engine
        for k in range(KT_H):
            nc.gpsimd.dma_start(out=yrows[k],
                                in_=yT_ap[k * 128:(k + 1) * 128, :])
        WVQ_DMA[0]()
        for k in range(KT_H):
            nc.sync.dma_start(out=xrows[k],
                              in_=xT_ap[k * 128:(k + 1) * 128, :])

        nc.sync.dma_start(out=wo_r,
                          in_=wo.ap().rearrange('(t p) n -> p t n', p=128))
        ones4 = res.tile([128, HPC, 1], F32, tag='ones4')
        nc.vector.memset(ones4, 1.0)

        # ---- resident activations ----
        QT = [res.tile([128, S], BF16, tag=f'qt{p}', name=f'qt{p}') for p in range(2)]
        KTs = [res.tile([128, S], BF16, tag=f'kt{p}', name=f'kt{p}') for p in range(2)]
        v_sb = [res.tile([128, HPC, DH + 1], BF16, tag=f'v{i}', name=f'v{i}')
                for i in range(SK)]

        # ---- projection groups (8 matmuls + eviction) ----
        gid = [0]

        def qk_group(which, p, j4):
            w_r = wq_r if which == 'q' else wk_r
            src = xts if which == 'q' else yts
            dest = QT[p] if which == 'q' else KTs[p]
            js = slice(j4 * JBLK, (j4 + 1) * JBLK)
            cs = slice(p * 128, (p + 1) * 128)
            gid[0] += 1
            gname = f'g{gid[0]}'
            box = {}
            items = []
            for k in range(KT_H):
                def mm(k=k):
                    if k == 0:
                        box['ps'] = ps_g.tile([128, JBLK], F32, tag='g',
                                              name=gname)
                    nc.tensor.matmul(box['ps'], w_r[:, k, cs], src[k][j4],
                                     start=(k == 0), stop=(k == KT_H - 1))
                items.append(mm)

            def fin():
                nc.vector.tensor_copy(dest[:, js], box['ps'])
            items.append(fin)
            return items

        def v_group(sub):
            j4, m = sub // 4, sub % 4
            ms = slice(m * 128, (m + 1) * 128)
            gid[0] += 1
            gname = f'g{gid[0]}'
            box = {}
            items = []
            for k in range(KT_H):
                def mm(k=k):
                    if k == 0:
                        box['ps'] = ps_g.tile([128, JBLK], F32, tag='g',
                                              name=gname)
                    nc.tensor.matmul(box['ps'][:, 0:C], yts[k][j4][:, ms],
                                     wv_r[:, k, :],
                                     start=(k == 0), stop=(k == KT_H - 1))
                items.append(mm)

            def fin():
                nc.vector.tensor_scalar_mul(
                    v_sb[sub][:, :, 0:DH],
                    box['ps'][:, 0:C].rearrange('p (h c) -> p h c', h=HPC),
                    eb[:, sub:sub + 1])
                nc.gpsimd.tensor_scalar_mul(v_sb[sub][:, :, DH:DH + 1], ones4,
                                            eb[:, sub:sub + 1])
            items.append(fin)
            return items

        # prefix: the minimum the first attention steps touch
        for grp in (qk_group('k', 0, 0), qk_group('q', 0, 0),
                    v_group(0), v_group(1)):
            for it in grp:
                it()

        # deadline-sorted weave of all remaining projection groups.
        # deadline = last global step at which the group may start so its
        # consumer (QK emission 2 steps ahead, or PV at step kk) is fed.
        entries = []
        for j4 in range(1, NJ):
            entries.append((max(0, 4 * j4 - 3), qk_group('k', 0, j4)))
        for sub in range(2, SK):
            entries.append((sub - 1, v_group(sub)))
        entries.append((8, qk_group('k', 1, 0)))
        entries.append((8, qk_group('q', 1, 0)))
        for j4 in range(1, NJ):
            entries.append((16 + 4 * j4 - 6, qk_group('k', 1, j4)))
        for Jq in range(1, NJ):
            entries.append((16 * (2 * Jq) - 6, qk_group('q', 0, Jq)))
            entries.append((16 * (2 * Jq + 1) - 6, qk_group('q', 1, Jq)))
        entries.sort(key=lambda e: e[0])
        weave = [[dl, items, 0] for dl, items in entries]  # [deadline, items, cursor]
        wpos = [0]

        def weave_left():
            return wpos[0] < len(weave)

        def emit_overdue(g):
            n = 0
            while weave_left() and weave[wpos[0]][0] <= g:
                ent = weave[wpos[0]]
                while ent[2] < len(ent[1]):
                    ent[1][ent[2]]()
                    ent[2] += 1
                    n += 1
                wpos[0] += 1
            return n

        def emit_greedy(k):
            n = 0
            while n < k and weave_left():
                ent = weave[wpos[0]]
                ent[1][ent[2]]()
                ent[2] += 1
                n += 1
                if ent[2] == len(ent[1]):
                    wpos[0] += 1
            return n

        # ---- attention + output projection ----
        pend = []

        def out_groups(J, ctx_tiles):
            groups = []
            for m in range(4):
                for n in range(2):
                    def grp(m=m, n=n):
                        ms = slice(m * 128, (m + 1) * 128)
                        ns = slice(n * JBLK, (n + 1) * JBLK)
                        pso = ps_g.tile([128, JBLK], F32, tag='g',
                                        name=f'o{J}_{m}_{n}')
                        for p in range(2):
                            nc.tensor.matmul(pso, ctx_tiles[p][:, ms],
                                             wo_r[:, p, ns],
                                             start=(p == 0), stop=(p == 1))
                        ob = outpool.tile([128, JBLK], F32, tag='ob')
                        nc.vector.tensor_copy(ob, pso)
                        nc.sync.dma_start(
                            out=out.ap()[J * JBLK + m * 128:
                                         J * JBLK + (m + 1) * 128, ns],
                            in_=ob)
                    groups.append(grp)
            return groups

        pairs = [(J, p) for J in range(NJ) for p in range(2)]
        psl_q = []

        def emit_qk(pidx, kk):
            if pidx >= len(pairs):
                return
            J, p = pairs[pidx]
            js = slice(J * JBLK, (J + 1) * JBLK)
            kks = slice(kk * 128, (kk + 1) * 128)
            psl = ps_qk.tile([128, 2 * JBLK], F32, tag='qk',
                             name=f'psl{pidx}_{kk}')
            nc.tensor.matmul(psl[:, 0:JBLK],
                             KTs[p][0:64, kks], QT[p][0:64, js],
                             start=True, stop=True, tile_position=(0, 0))
            nc.tensor.matmul(psl[:, JBLK:2 * JBLK],
                             KTs[p][64:128, kks], QT[p][64:128, js],
                             start=True, stop=True, tile_position=(64, 0))
            psl_q.append(psl)

        emit_qk(0, 0)
        emit_qk(0, 1)
        for J in range(NJ):
            js = slice(J * JBLK, (J + 1) * JBLK)
            ctx_tiles = []
            for p in range(2):
                pidx = J * 2 + p
                pv0 = ps_acc.tile([128, JBLK], F32, tag='acc')
                pv1 = ps_acc.tile([128, JBLK], F32, tag='acc')
                for kk in range(SK):
                    g = pidx * SK + kk
                    # QK two steps ahead, crossing pair boundaries
                    if kk + 2 < SK:
                        emit_qk(pidx, kk + 2)
                    else:
                        emit_qk(pidx + 1, kk + 2 - SK)
                    psl = psl_q.pop(0)
                    emitted = emit_overdue(g)
                    if not emitted:
                        if pend and (pidx < 6 or (pidx == 6 and kk < 4)) and (
                                kk % 2 == 0 or not weave_left()):
                            pend.pop(0)()
                        else:
                            emit_greedy(2)
                    ex = expool.tile([128, 2 * JBLK], BF16, tag='ex')
                    nc.scalar.activation(ex, psl,
                                         mybir.ActivationFunctionType.Exp)
                    if _DEBUG and pidx == 0 and kk == 0:
                        de = outpool.tile([128, 2 * JBLK], F32, tag='de',
                                          bufs=1)
                        nc.vector.tensor_copy(de, ex)
                        nc.sync.dma_start(out=dbg['d_ex'].ap(), in_=de)
                    for hh, pv in enumerate((pv0, pv1)):
                        hcol = 2 * p + hh
                        nc.tensor.matmul(
                            pv[0:DH + 1, :],
                            v_sb[kk][:, hcol, :],
                            ex[:, hh * JBLK:(hh + 1) * JBLK],
                            start=(kk == 0), stop=(kk == SK - 1))
                # normalize: ctxT[d, q] * (1/denom[q]) via partition broadcast
                ct = ctxpool.tile([128, JBLK], BF16, tag=f'ctx{p}')
                stage = []
                for hh, pv in enumerate((pv0, pv1)):
                    rawct = small.tile([128, JBLK], F32, tag='rawct')
                    nc.vector.tensor_copy(rawct[0:DH + 1, :], pv[0:DH + 1, :])
                    rec = small.tile([128, JBLK], F32, tag='rec')
                    nc.vector.reciprocal_approx_fast(rec[0:DH + 1, :],
                                                     rawct[0:DH + 1, :])
                    bcs = small.tile([128, JBLK], F32, tag='bcs')
                    nc.sync.dma_start(out=bcs[0:1, :], in_=rec[DH:DH + 1, :])
                    bc = small.tile([128, JBLK], F32, tag='bc')
                    nc.gpsimd.partition_broadcast(bc[0:DH, :], bcs[0:1, :])
                    stage.append((rawct, bc))
                for hh, (rawct, bc) in enumerate(stage):
                    if hh == 0:
                        nc.vector.tensor_mul(ct[0:DH, :], rawct[0:DH, :],
                                             bc[0:DH, :])
                    else:
                        tmp = small.tile([128, JBLK], BF16, tag='tmp')
                        nc.vector.tensor_mul(tmp[0:DH, :], rawct[0:DH, :],
                                             bc[0:DH, :])
                        nc.sync.dma_start(out=ct[DH:128, :], in_=tmp[0:DH, :])
                if _DEBUG and pidx == 0:
                    dc = outpool.tile([128, JBLK], F32, tag='dc', bufs=1)
                    nc.vector.tensor_copy(dc, ct)
                    nc.sync.dma_start(out=dbg['d_ctx'].ap(), in_=dc)
                    dr = outpool.tile([128, JBLK], F32, tag='dr', bufs=1)
                    nc.vector.tensor_copy(dr[0:DH + 1, :],
                                          stage[0][0][0:DH + 1, :])
                    nc.sync.dma_start(out=dbg['d_raw'].ap(), in_=dr)
                ctx_tiles.append(ct)
            for grp in pend:       # drain any leftovers before replacing
                grp()
            pend = out_groups(J, ctx_tiles)
        while weave_left():
            emit_greedy(9)
        for grp in pend:
            grp()
        if _DEBUG:
            for nm, src, w in [('d_kt', KTs[0], S), ('d_qt', QT[0], S),
                               ('d_v', v_sb[0].rearrange('p h c -> p (h c)'),
                                HPC * (DH + 1)),
                               ('d_wo', wo_r.rearrange('p t n -> p (t n)'),
                                2 * H)]:
                dd = res.tile([128, w], F32, tag=f'dd{nm}', name=f'dd{nm}')
                nc.vector.tensor_copy(dd, src)
                nc.sync.dma_start(out=dbg[nm].ap(), in_=dd)

    nc.compile()
    return nc


def _get_nc():
    if 'nc' not in _CACHE:
        _CACHE['nc'] = _build()
    return _CACHE['nc']


def shard_inputs(x, y, bias, wq, wk, wv, wo):
    """Build the 8 per-core input maps from full inputs."""
    scale = (H // NH) ** -0.5
    wqs = (wq * scale).astype(np.float32)
    bf = ml_dtypes.bfloat16
    in_maps = []
    for c in range(NCORES):
        b = c // (NCORES // B)
        g = c % (NCORES // B)
        cols = slice(g * C, (g + 1) * C)
        eb = np.exp(bias[b, 0, 0, :].astype(np.float64)).astype(np.float32)
        in_maps.append({
            'xT': np.ascontiguousarray(x[b].T.astype(bf)),
            'yT': np.ascontiguousarray(y[b].T.astype(bf)),
            'wq': np.ascontiguousarray(wqs[:, cols].astype(bf)),
            'wk': np.ascontiguousarray(wk[:, cols].astype(bf)),
            'wv': np.ascontiguousarray(wv[:, cols].astype(bf)),
            'wo': np.ascontiguousarray(wo[cols, :].astype(bf)),
            'ebias': np.ascontiguousarray(eb.reshape(SK, 128).T),
        })
    return in_maps


def kernel(x, y, bias, wq, wk, wv, wo, _trace=False):
    x, y, bias = np.asarray(x), np.asarray(y), np.asarray(bias)
    wq, wk, wv, wo = (np.asarray(t) for t in (wq, wk, wv, wo))
    nc = _get_nc()
    in_maps = shard_inputs(x, y, bias, wq, wk, wv, wo)
    kw = {}
    if _trace:
        kw = dict(trace=True, stitch_traces=False)
    res = bass_utils.run_bass_kernel_spmd(nc, in_maps, core_ids=list(range(NCORES)), **kw)
    full = np.zeros((B, S, H), dtype=np.float64)
    for c in range(NCORES):
        full[c // (NCORES // B)] += res.results[c]['out'].astype(np.float64)
    if _trace:
        _CACHE['last_results'] = res
    return full.astype(np.float32)


# revision 19
# speedup vs baseline: 1.1059x; 1.1059x over previous
"""Multi-head attention (B=2, S=2048, H=1024, 16 heads) on 8 TRN2 NeuronCores.

Sharding: data parallel on batch (2) x tensor parallel on heads (4 heads/core,
Megatron column-split qkv, row-split wo). Host pre-transposes x/y, pre-scales
wq by dh^-0.5, and sum-reduces the 4 partial outputs per batch element.

Per-core kernel (v2):
  The attention loop is ACT(exp)-paced at ~1.1us/step (128 steps). All
  projection work beyond a minimal prefix (K/Q j-block 0 + first two V
  subtiles) is woven into the loop via a deadline-sorted schedule, so the
  prefix shrinks from ~59us to ~12us. Input DMA is chunked [128,512] and
  ordered so the first projection group can start as chunks land; dummy
  warmup matmuls ramp the PE p-state during the initial DMA wait. Output
  stays f32 (bf16 DRAM outputs do not land on HW).

  Attention per 512-wide q-block and head pair: row-tiled (2-head packed)
  QK^T -> logitsT psum [128,1024] -> one ACT exp per pair (psum->sbuf bf16)
  -> PV matmul with fused denominator row (fp32 accumulate) -> fast
  reciprocal + gpsimd partition_broadcast normalize -> pair-stacked bf16
  output projection, deferred one block for overlap.
"""
import sys
sys.path.insert(0, '/opt/trn_rl_repo')
from contextlib import ExitStack

import numpy as np
import ml_dtypes

import concourse.bacc as bacc
import concourse.tile as tile
from concourse import mybir
from concourse import bass_utils

B, S, H, NH = 2, 2048, 1024, 16
DH = H // NH            # 64
NCORES = 8
HPC = NH // (NCORES // B)   # 4 heads per core
C = HPC * DH            # 256 projected cols per core
KT_H = H // 128         # 8 contraction tiles over H
SK = S // 128           # 16 s-subtiles
JBLK = 512
NJ = S // JBLK          # 4 q-blocks
F32 = mybir.dt.float32
BF16 = mybir.dt.bfloat16

_CACHE = {}
_DEBUG = False


def _build():
    nc = bacc.Bacc('TRN2', debug=False, num_devices=NCORES)
    xT = nc.dram_tensor('xT', [H, S], BF16, kind='ExternalInput')
    yT = nc.dram_tensor('yT', [H, S], BF16, kind='ExternalInput')
    wq = nc.dram_tensor('wq', [H, C], BF16, kind='ExternalInput')
    wk = nc.dram_tensor('wk', [H, C], BF16, kind='ExternalInput')
    wv = nc.dram_tensor('wv', [H, C], BF16, kind='ExternalInput')
    wo = nc.dram_tensor('wo', [C, H], BF16, kind='ExternalInput')
    ebias = nc.dram_tensor('ebias', [128, SK], F32, kind='ExternalInput')
    out = nc.dram_tensor('out', [S, H], F32, kind='ExternalOutput')
    dbg = {}
    if _DEBUG:
        for nm, w in [('d_kt', S), ('d_qt', S), ('d_ex', 2 * JBLK),
                      ('d_v', HPC * (DH + 1)), ('d_wo', 2 * H),
                      ('d_ctx', JBLK), ('d_raw', JBLK)]:
            dbg[nm] = nc.dram_tensor(nm, [128, w], F32, kind='ExternalOutput')

    with tile.TileContext(nc) as tc, ExitStack() as ctx:
        res = ctx.enter_context(tc.tile_pool(name='res', bufs=1))
        expool = ctx.enter_context(tc.tile_pool(name='expool', bufs=4))
        ctxpool = ctx.enter_context(tc.tile_pool(name='ctxpool', bufs=2))
        small = ctx.enter_context(tc.tile_pool(name='small', bufs=3))
        outpool = ctx.enter_context(tc.tile_pool(name='outpool', bufs=3))
        ps_qk = ctx.enter_context(tc.tile_pool(name='ps_qk', bufs=2, space='PSUM'))
        ps_acc = ctx.enter_context(tc.tile_pool(name='ps_acc', bufs=2, space='PSUM'))
        ps_g = ctx.enter_context(tc.tile_pool(name='ps_g', bufs=2, space='PSUM'))

        # ---- input DMAs, ordered for earliest compute start ----
        eb = res.tile([128, SK], F32, tag='eb')
        wk_r = res.tile([128, KT_H, C], BF16, tag='wk')
        wv_r = res.tile([128, KT_H, C], BF16, tag='wv')
        wq_r = res.tile([128, KT_H, C], BF16, tag='wq')
        wo_r = res.tile([128, 2, H], BF16, tag='wo')
        nc.sync.dma_start(out=wk_r, in_=wk.ap().rearrange('(t p) c -> p t c', p=128))
        nc.sync.dma_start(out=eb, in_=ebias.ap())
        WVQ_DMA = [lambda: (
            nc.sync.dma_start(out=wv_r,
                              in_=wv.ap().rearrange('(t p) c -> p t c', p=128)),
            nc.sync.dma_start(out=wq_r,
                              in_=wq.ap().rearrange('(t p) c -> p t c', p=128)))]

        xrows = [res.tile([128, S], BF16, tag=f'xr{k}', name=f'xr{k}')
                 for k in range(KT_H)]
        yrows = [res.tile([128, S], BF16, tag=f'yr{k}', name=f'yr{k}')
                 for k in range(KT_H)]
        xts = [[xrows[k][:, j * JBLK:(j + 1) * JBLK] for j in range(NJ)]
               for k in range(KT_H)]
        yts = [[yrows[k][:, j * JBLK:(j + 1) * JBLK] for j in range(NJ)]
               for k in range(KT_H)]
        xT_ap, yT_ap = xT.ap(), yT.ap()

        # each dma_start costs ~600ns of ISSUING-sequencer time, and each
        # dma_start round-robins to one of 16 HW queues. Everything goes on
        # sync (ACT must stay free for EXPs) in strict priority order:
        # fine-grained chunks for data needed in the first attention pair,
        # one coarse chunk for the late-needed x remainder.
        def dma_chunk(tiles, src_ap, j):
            for k in range(KT_H):
                nc.sync.dma_start(
                    out=tiles[k][:, j * JBLK:(j + 1) * JBLK],
                    in_=src_ap[k * 128:(k + 1) * 128,
                               j * JBLK:(j + 1) * JBLK])

        dma_chunk(yrows, yT_ap, 0)
        WVQ_DMA[0]()
        dma_chunk(xrows, xT_ap, 0)
        dma_chunk(yrows, yT_ap, 1)
        dma_chunk(yrows, yT_ap, 2)
        dma_chunk(yrows, yT_ap, 3)
        for k in range(KT_H):
            nc.sync.dma_start(out=xrows[k][:, JBLK:],
                              in_=xT_ap[k * 128:(k + 1) * 128, JBLK:])

        nc.sync.dma_start(out=wo_r,
                          in_=wo.ap().rearrange('(t p) n -> p t n', p=128))
        ones4 = res.tile([128, HPC, 1], F32, tag='ones4')
        nc.vector.memset(ones4, 1.0)

        # ---- resident activations ----
        QT = [res.tile([128, S], BF16, tag=f'qt{p}', name=f'qt{p}') for p in range(2)]
        KTs = [res.tile([128, S], BF16, tag=f'kt{p}', name=f'kt{p}') for p in range(2)]
        v_sb = [res.tile([128, HPC, DH + 1], BF16, tag=f'v{i}', name=f'v{i}')
                for i in range(SK)]

        # ---- projection groups (8 matmuls + eviction) ----
        gid = [0]

        def qk_group(which, p, j4):
            w_r = wq_r if which == 'q' else wk_r
            src = xts if which == 'q' else yts
            dest = QT[p] if which == 'q' else KTs[p]
            js = slice(j4 * JBLK, (j4 + 1) * JBLK)
            cs = slice(p * 128, (p + 1) * 128)
            gid[0] += 1
            gname = f'g{gid[0]}'
            box = {}
            items = []
            for k in range(KT_H):
                def mm(k=k):
                    if k == 0:
                        box['ps'] = ps_g.tile([128, JBLK], F32, tag='g',
                                              name=gname)
                    nc.tensor.matmul(box['ps'], w_r[:, k, cs], src[k][j4],
                                     start=(k == 0), stop=(k == KT_H - 1))
                items.append(mm)

            def fin():
                nc.vector.tensor_copy(dest[:, js], box['ps'])
            items.append(fin)
            return items

        def v_group(sub):
            j4, m = sub // 4, sub % 4
            ms = slice(m * 128, (m + 1) * 128)
            gid[0] += 1
            gname = f'g{gid[0]}'
            box = {}
            items = []
            for k in range(KT_H):
                def mm(k=k):
                    if k == 0:
                        box['ps'] = ps_g.tile([128, JBLK], F32, tag='g',
                                              name=gname)
                    nc.tensor.matmul(box['ps'][:, 0:C], yts[k][j4][:, ms],
                                     wv_r[:, k, :],
                                     start=(k == 0), stop=(k == KT_H - 1))
                items.append(mm)

            def fin():
                nc.vector.tensor_scalar_mul(
                    v_sb[sub][:, :, 0:DH],
                    box['ps'][:, 0:C].rearrange('p (h c) -> p h c', h=HPC),
                    eb[:, sub:sub + 1])
                nc.gpsimd.tensor_scalar_mul(v_sb[sub][:, :, DH:DH + 1], ones4,
                                            eb[:, sub:sub + 1])
            items.append(fin)
            return items

        # prefix: the minimum the first attention steps touch
        for grp in (qk_group('k', 0, 0), qk_group('q', 0, 0),
                    v_group(0), v_group(1)):
            for it in grp:
                it()

        # deadline-sorted weave of all remaining projection groups.
        # deadline = last global step at which the group may start so its
        # consumer (QK emission 2 steps ahead, or PV at step kk) is fed.
        entries = []
        for j4 in range(1, NJ):
            entries.append((max(0, 4 * j4 - 3), qk_group('k', 0, j4)))
        for sub in range(2, SK):
            entries.append((sub - 1, v_group(sub)))
        entries.append((8, qk_group('k', 1, 0)))
        entries.append((8, qk_group('q', 1, 0)))
        for j4 in range(1, NJ):
            entries.append((16 + 4 * j4 - 6, qk_group('k', 1, j4)))
        for Jq in range(1, NJ):
            entries.append((16 * (2 * Jq) - 6, qk_group('q', 0, Jq)))
            entries.append((16 * (2 * Jq + 1) - 6, qk_group('q', 1, Jq)))
        entries.sort(key=lambda e: e[0])
        weave = [[dl, items, 0] for dl, items in entries]  # [deadline, items, cursor]
        wpos = [0]

        def weave_left():
            return wpos[0] < len(weave)

        def emit_overdue(g):
            n = 0
            while weave_left() and weave[wpos[0]][0] <= g:
                ent = weave[wpos[0]]
                while ent[2] < len(ent[1]):
                    ent[1][ent[2]]()
                    ent[2] += 1
                    n += 1
                wpos[0] += 1
            return n

        def emit_greedy(k):
            n = 0
            while n < k and weave_left():
                ent = weave[wpos[0]]
                ent[1][ent[2]]()
                ent[2] += 1
                n += 1
                if ent[2] == len(ent[1]):
                    wpos[0] += 1
            return n

        # ---- attention + output projection ----
        pend = []

        def out_groups(J, ctx_tiles):
            groups = []
            for m in range(4):
                for n in range(2):
                    def grp(m=m, n=n):
                        ms = slice(m * 128, (m + 1) * 128)
                        ns = slice(n * JBLK, (n + 1) * JBLK)
                        pso = ps_g.tile([128, JBLK], F32, tag='g',
                                        name=f'o{J}_{m}_{n}')
                        for p in range(2):
                            nc.tensor.matmul(pso, ctx_tiles[p][:, ms],
                                             wo_r[:, p, ns],
                                             start=(p == 0), stop=(p == 1))
                        ob = outpool.tile([128, JBLK], F32, tag='ob')
                        nc.vector.tensor_copy(ob, pso)
                        nc.sync.dma_start(
                            out=out.ap()[J * JBLK + m * 128:
                                         J * JBLK + (m + 1) * 128, ns],
                            in_=ob)
                    groups.append(grp)
            return groups

        pairs = [(J, p) for J in range(NJ) for p in range(2)]
        psl_q = []

        def emit_qk(pidx, kk):
            if pidx >= len(pairs):
                return
            J, p = pairs[pidx]
            js = slice(J * JBLK, (J + 1) * JBLK)
            kks = slice(kk * 128, (kk + 1) * 128)
            psl = ps_qk.tile([128, 2 * JBLK], F32, tag='qk',
                             name=f'psl{pidx}_{kk}')
            nc.tensor.matmul(psl[:, 0:JBLK],
                             KTs[p][0:64, kks], QT[p][0:64, js],
                             start=True, stop=True, tile_position=(0, 0))
            nc.tensor.matmul(psl[:, JBLK:2 * JBLK],
                             KTs[p][64:128, kks], QT[p][64:128, js],
                             start=True, stop=True, tile_position=(64, 0))
            psl_q.append(psl)

        emit_qk(0, 0)
        emit_qk(0, 1)
        for J in range(NJ):
            js = slice(J * JBLK, (J + 1) * JBLK)
            ctx_tiles = []
            for p in range(2):
                pidx = J * 2 + p
                pv0 = ps_acc.tile([128, JBLK], F32, tag='acc')
                pv1 = ps_acc.tile([128, JBLK], F32, tag='acc')
                for kk in range(SK):
                    g = pidx * SK + kk
                    # QK two steps ahead, crossing pair boundaries
                    if kk + 2 < SK:
                        emit_qk(pidx, kk + 2)
                    else:
                        emit_qk(pidx + 1, kk + 2 - SK)
                    psl = psl_q.pop(0)
                    emitted = emit_overdue(g)
                    if not emitted:
                        if pend and (pidx < 6 or (pidx == 6 and kk < 4)) and (
                                kk % 2 == 0 or not weave_left()):
                            pend.pop(0)()
                        else:
                            emit_greedy(2)
                    ex = expool.tile([128, 2 * JBLK], BF16, tag='ex')
                    nc.scalar.activation(ex, psl,
                                         mybir.ActivationFunctionType.Exp)
                    if _DEBUG and pidx == 0 and kk == 0:
                        de = outpool.tile([128, 2 * JBLK], F32, tag='de',
                                          bufs=1)
                        nc.vector.tensor_copy(de, ex)
                        nc.sync.dma_start(out=dbg['d_ex'].ap(), in_=de)
                    for hh, pv in enumerate((pv0, pv1)):
                        hcol = 2 * p + hh
                        nc.tensor.matmul(
                            pv[0:DH + 1, :],
                            v_sb[kk][:, hcol, :],
                            ex[:, hh * JBLK:(hh + 1) * JBLK],
                            start=(kk == 0), stop=(kk == SK - 1))
                # normalize: ctxT[d, q] * (1/denom[q]) via partition broadcast
                ct = ctxpool.tile([128, JBLK], BF16, tag=f'ctx{p}')
                stage = []
                for hh, pv in enumerate((pv0, pv1)):
                    rawct = small.tile([128, JBLK], F32, tag='rawct')
                    nc.vector.tensor_copy(rawct[0:DH + 1, :], pv[0:DH + 1, :])
                    rec = small.tile([128, JBLK], F32, tag='rec')
                    nc.vector.reciprocal_approx_fast(rec[0:DH + 1, :],
                                                     rawct[0:DH + 1, :])
                    bcs = small.tile([128, JBLK], F32, tag='bcs')
                    nc.sync.dma_start(out=bcs[0:1, :], in_=rec[DH:DH + 1, :])
                    bc = small.tile([128, JBLK], F32, tag='bc')
                    nc.gpsimd.partition_broadcast(bc[0:DH, :], bcs[0:1, :])
                    stage.append((rawct, bc))
                for hh, (rawct, bc) in enumerate(stage):
                    if hh == 0:
                        nc.vector.tensor_mul(ct[0:DH, :], rawct[0:DH, :],
                                             bc[0:DH, :])
                    else:
                        tmp = small.tile([128, JBLK], BF16, tag='tmp')
                        nc.vector.tensor_mul(tmp[0:DH, :], rawct[0:DH, :],
                                             bc[0:DH, :])
                        nc.sync.dma_start(out=ct[DH:128, :], in_=tmp[0:DH, :])
                if _DEBUG and pidx == 0:
                    dc = outpool.tile([128, JBLK], F32, tag='dc', bufs=1)
                    nc.vector.tensor_copy(dc, ct)
                    nc.sync.dma_start(out=dbg['d_ctx'].ap(), in_=dc)
                    dr = outpool.tile([128, JBLK], F32, tag='dr', bufs=1)
                    nc.vector.tensor_copy(dr[0:DH + 1, :],
                                          stage[0][0][0:DH + 1, :])
                    nc.sync.dma_start(out=dbg['d_raw'].ap(), in_=dr)
                ctx_tiles.append(ct)
            for grp in pend:       # drain any leftovers before replacing
                grp()
            pend = out_groups(J, ctx_tiles)
        while weave_left():
            emit_greedy(9)
        for grp in pend:
            grp()
        if _DEBUG:
            for nm, src, w in [('d_kt', KTs[0], S), ('d_qt', QT[0], S),
                               ('d_v', v_sb[0].rearrange('p h c -> p (h c)'),
                                HPC * (DH + 1)),
                               ('d_wo', wo_r.rearrange('p t n -> p (t n)'),
                                2 * H)]:
                dd = res.tile([128, w], F32, tag=f'dd{nm}', name=f'dd{nm}')
                nc.vector.tensor_copy(dd, src)
                nc.sync.dma_start(out=dbg[nm].ap(), in_=dd)

    nc.compile()
    return nc


def _get_nc():
    if 'nc' not in _CACHE:
        _CACHE['nc'] = _build()
    return _CACHE['nc']


def shard_inputs(x, y, bias, wq, wk, wv, wo):
    """Build the 8 per-core input maps from full inputs."""
    scale = (H // NH) ** -0.5
    wqs = (wq * scale).astype(np.float32)
    bf = ml_dtypes.bfloat16
    in_maps = []
    for c in range(NCORES):
        b = c // (NCORES // B)
        g = c % (NCORES // B)
        cols = slice(g * C, (g + 1) * C)
        eb = np.exp(bias[b, 0, 0, :].astype(np.float64)).astype(np.float32)
        in_maps.append({
            'xT': np.ascontiguousarray(x[b].T.astype(bf)),
            'yT': np.ascontiguousarray(y[b].T.astype(bf)),
            'wq': np.ascontiguousarray(wqs[:, cols].astype(bf)),
            'wk': np.ascontiguousarray(wk[:, cols].astype(bf)),
            'wv': np.ascontiguousarray(wv[:, cols].astype(bf)),
            'wo': np.ascontiguousarray(wo[cols, :].astype(bf)),
            'ebias': np.ascontiguousarray(eb.reshape(SK, 128).T),
        })
    return in_maps


def kernel(x, y, bias, wq, wk, wv, wo, _trace=False):
    x, y, bias = np.asarray(x), np.asarray(y), np.asarray(bias)
    wq, wk, wv, wo = (np.asarray(t) for t in (wq, wk, wv, wo))
    nc = _get_nc()
    in_maps = shard_inputs(x, y, bias, wq, wk, wv, wo)
    kw = {}
    if _trace:
        kw = dict(trace=True, stitch_traces=False)
    res = bass_utils.run_bass_kernel_spmd(nc, in_maps, core_ids=list(range(NCORES)), **kw)
    full = np.zeros((B, S, H), dtype=np.float64)
    for c in range(NCORES):
        full[c // (NCORES // B)] += res.results[c]['out'].astype(np.float64)
    if _trace:
        _CACHE['last_results'] = res
    return full.astype(np.float32)
